# revision 24
# baseline (speedup 1.0000x reference)
"""Patch-embedding kernel for Trainium2, data-parallel over batch on 8 NeuronCores.

Reference computation (per image):
  patches = im2col(image, 16x16)            # [196, 768]
  out = gelu(patches @ W + b, exact)        # [196, 768]

Sharding: batch 64 -> 8 images per core; host concatenates per-core outputs.

Layout strategy: im2col is a pure permutation for stride-16 non-overlapping
patches, so the HOST performs im2col + transpose + bf16 cast and uploads
X^T in k-major chunk layout xt[p, kc, m] = X[m, 128*kc + p]. Every device
DMA is then a wide contiguous read (>=1.5 KB per partition line) -- this
removes the 192-byte-fragment im2col gather and the on-device xbar
transposes that dominated the previous version.

Matmul orientation: transposed output. For each 128-wide n-chunk,
  psum[n, m] = sum_kc W[k, n].T @ X^T[k, m]
with W chunks as the stationary operand (natural layout, uploaded
pre-chunked) and X^T as the bf16 moving operand. Benefits:
  - no bias matmuls: bias is per-PARTITION in this orientation, applied for
    free by ScalarE as gelu(psum + bias[p]) during the PSUM->SBUF pass
  - m-tiles of 392 (=1568/4) tile M exactly; lhsT is always full 128x128
  - output stored bf16 (halves store traffic); host transposes + upcasts.

Per-core schedule (m-blocks of 196/392/392/392/196):
  - Loads ride ONE HWDGE ring in exact consumption order (both rings
    share the 16 SDMA engines, so a second ring adds no bandwidth);
    the first m-block's W/X chunks are interleaved so the PE starts
    ~2 us after loads begin and paces with the DMA stream.
  - A few matmuls on a zeroed tile bridge the PE from the entry barrier
    to the first real matmul so the HAM clock gate (cold 1.2 GHz ->
    warm 2.4 GHz after ~3.4 us of sustained busy) warms during loads.
  - First m-block runs kc-outer with all 6 n-chunk PSUM groups live so
    each arriving chunk pair immediately yields 6 matmuls; later blocks
    run n6-outer, one PSUM bank per group, rotating through all 8 banks.
  - ScalarE applies exact GELU (+ per-partition bias) PSUM->SBUF bf16;
    stores overlap compute on the sync ring.
Steady-state matmul cadence measured at the 166 ns streaming floor for
N=392 bf16 (LDWEIGHTS fully hidden).
"""

import numpy as np
import ml_dtypes

import concourse.bass as bass
import concourse.tile as tile
import concourse.mybir as mybir
from concourse import bacc
from concourse.bass_utils import run_bass_kernel_spmd

P = 16
D = 768
B, H, W, C = 64, 224, 224, 3
NH = NW = 14
NPATCH = NH * NW            # 196
K = P * P * C               # 768
NCORES = 8
BPC = B // NCORES           # 8 images per core
M = BPC * NPATCH            # 1568 output rows per core
KC = K // 128               # 6 k-chunks
NC6 = D // 128              # 6 n-chunks
MT = 392                    # interior m-tile width
# m-block split: small first block so the PE can start on ~1.5 MB of
# loads instead of ~1.8, and a small last block to shorten the tail.
MBLOCKS = (196, 392, 392, 392, 196)
MOFF = (0, 196, 588, 980, 1372)

_BF16 = mybir.dt.bfloat16
_F32 = mybir.dt.float32


def _build():
    nc = bacc.Bacc("TRN2", target_bir_lowering=False, debug=False,
                   num_devices=NCORES)
    # Host-prepared layouts (see _run): all reads/writes contiguous.
    xt = nc.dram_tensor("xt", [128, KC, M], _BF16, kind="ExternalInput").ap()
    # Block0's X as its own contiguous tensor: the strided [., kc, 0:196]
    # slices of xt are 392-B runs, below the 512-B SDMA line-rate
    # threshold (read-modify-write on the SBUF side); this loads as one
    # DMA with 2352-B runs instead of two RMW-penalized ones.
    xt0 = nc.dram_tensor("xt0", [128, KC, MBLOCKS[0]], _BF16,
                         kind="ExternalInput").ap()
    w = nc.dram_tensor("w", [128, KC, D], _BF16, kind="ExternalInput").ap()
    bias = nc.dram_tensor("bias", [128, NC6], _F32, kind="ExternalInput").ap()
    # Transposed output out[p, n6, m] = result[m, 128*n6 + p]; host unscrambles.
    out = nc.dram_tensor("out", [128, NC6, M], _BF16, kind="ExternalOutput").ap()

    with tile.TileContext(nc) as tc:
        _body(tc, xt, xt0, w, bias, out)
    nc.compile()
    return nc


def _body(tc, xt, xt0, w, bias, out):
    import contextlib
    ctx = contextlib.ExitStack()
    with ctx:
        nc = tc.nc
        singles = ctx.enter_context(tc.tile_pool(name="singles", bufs=1))
        opool = ctx.enter_context(tc.tile_pool(name="o", bufs=4))
        pspool = ctx.enter_context(tc.tile_pool(name="ps", bufs=8, space="PSUM"))

        # Loads on the sync ring in exact consumption order. The first
        # m-block's dependencies (W + 0.3 MB of X^T) are interleaved so
        # the kc-outer loop below consumes chunk k while chunk k+1 is in
        # flight; later blocks stream while the PE chews. bias goes on
        # the scalar ring (only needed by the first activation).
        bias_sb = singles.tile([128, NC6], _F32)
        nc.scalar.dma_start(out=bias_sb[:], in_=bias[:])
        w_sb = singles.tile([128, KC, D], _BF16)
        xt_sb = singles.tile([128, KC, M], _BF16)
        B0 = MBLOCKS[0]
        xt0_sb = singles.tile([128, KC, B0], _BF16)
        nc.sync.dma_start(out=w_sb[:, 0:2, :], in_=w[:, 0:2, :])
        nc.sync.dma_start(out=xt0_sb[:], in_=xt0[:])
        nc.sync.dma_start(out=w_sb[:, 2:4, :], in_=w[:, 2:4, :])
        nc.sync.dma_start(out=w_sb[:, 4:6, :], in_=w[:, 4:6, :])
        for mb in range(1, len(MBLOCKS)):
            sl = np.s_[:, :, MOFF[mb]:MOFF[mb] + MBLOCKS[mb]]
            nc.sync.dma_start(out=xt_sb[sl], in_=xt[sl])

        # Matmuls on a zeroed tile bridge the PE continuously from the
        # entry barrier to the first real matmul so the HAM clock gate's
        # ~3.4 us busy window elapses during the load phase instead of
        # during real work.
        zeros = singles.tile([128, 384], _BF16)
        nc.vector.memset(zeros[:], 0.0)
        ps_warm = pspool.tile([128, 512], _F32, tag="ps")
        for _ in range(9):
            nc.tensor.matmul(ps_warm[:, :384], zeros[:, :128], zeros[:],
                             start=True, stop=True)

        # First m-block: kc-outer with all 6 n-chunk PSUM groups live, so
        # each arriving (w, x) chunk pair immediately yields 6 matmuls.
        ps0 = [pspool.tile([128, 512], _F32, tag="ps", name=f"ps0_{i}")
               for i in range(NC6)]
        for kc in range(KC):
            for n6 in range(NC6):
                nc.tensor.matmul(ps0[n6][:, :B0],
                                 w_sb[:, kc, n6 * 128:(n6 + 1) * 128],
                                 xt0_sb[:, kc, :],
                                 start=(kc == 0), stop=(kc == KC - 1))
        for n6 in range(NC6):
            o_sb = opool.tile([128, B0], _BF16, tag="o0")
            nc.scalar.activation(o_sb[:], ps0[n6][:, :B0],
                                 mybir.ActivationFunctionType.Gelu,
                                 bias=bias_sb[:, n6:n6 + 1])
            nc.sync.dma_start(out=out[:, n6, 0:B0], in_=o_sb[:])

        # Remaining m-blocks: data is resident (or lands just ahead);
        # n6-outer keeps the activation/store pipeline finely paced.
        for mb in range(1, len(MBLOCKS)):
            m0, mw = MOFF[mb], MBLOCKS[mb]
            for n6 in range(NC6):
                ps = pspool.tile([128, 512], _F32, tag="ps")
                for kc in range(KC):
                    nc.tensor.matmul(ps[:, :mw],
                                     w_sb[:, kc, n6 * 128:(n6 + 1) * 128],
                                     xt_sb[:, kc, m0:m0 + mw],
                                     start=(kc == 0), stop=(kc == KC - 1))
                o_sb = opool.tile([128, mw], _BF16, tag=f"o{mw}")
                nc.scalar.activation(o_sb[:], ps[:, :mw],
                                     mybir.ActivationFunctionType.Gelu,
                                     bias=bias_sb[:, n6:n6 + 1])
                nc.sync.dma_start(out=out[:, n6, m0:m0 + mw], in_=o_sb[:])


_NC_CACHE = {}


def _get_nc():
    if "nc" not in _NC_CACHE:
        _NC_CACHE["nc"] = _build()
    return _NC_CACHE["nc"]


def _prep_core_inputs(image, W_proj, b_proj):
    """Host-side layout prep: im2col + transpose + bf16, all permutations."""
    image = np.asarray(image, dtype=np.float32)
    assert image.shape == (B, H, W, C)
    img_bf = image.astype(ml_dtypes.bfloat16)
    # im2col (row-major patch order, matching the reference)
    pat = img_bf.reshape(B, NH, P, NW, P, C).transpose(0, 1, 3, 2, 4, 5)
    pat = np.ascontiguousarray(pat).reshape(B, NPATCH, K)

    w_bf = np.asarray(W_proj, dtype=np.float32).astype(ml_dtypes.bfloat16)
    w_dev = np.ascontiguousarray(w_bf.reshape(KC, 128, D).transpose(1, 0, 2))
    b_dev = np.ascontiguousarray(
        np.asarray(b_proj, dtype=np.float32).reshape(NC6, 128).T)

    in_maps = []
    for c in range(NCORES):
        x = pat[c * BPC:(c + 1) * BPC].reshape(M, K)
        # xt[p, kc, m] = x[m, 128*kc + p]
        xt = np.ascontiguousarray(x.reshape(M, KC, 128).transpose(2, 1, 0))
        xt0 = np.ascontiguousarray(xt[:, :, 0:MBLOCKS[0]])
        in_maps.append({"xt": xt, "xt0": xt0, "w": w_dev, "bias": b_dev})
    return in_maps


def _run(image, W_proj, b_proj, **spmd_kwargs):
    spmd_kwargs.pop("transpose_mode", None)
    in_maps = _prep_core_inputs(image, W_proj, b_proj)
    nc = _get_nc()
    res = run_bass_kernel_spmd(nc, in_maps, core_ids=list(range(NCORES)),
                               **spmd_kwargs)
    # device layout [p, n6, m] -> [m, 128*n6+p] -> [BPC, NPATCH, D] f32
    outs = [
        np.ascontiguousarray(res.results[c]["out"].transpose(2, 1, 0))
        .astype(np.float32).reshape(BPC, NPATCH, D)
        for c in range(NCORES)
    ]
    full = np.concatenate(outs, axis=0)
    return full, res


def kernel(image, W_proj, b_proj):
    full, _ = _run(image, W_proj, b_proj)
    return full


# revision 30
# speedup vs baseline: 1.0353x; 1.0353x over previous
"""Patch-embedding kernel for Trainium2, data-parallel over batch on 8 NeuronCores.

Reference computation (per image):
  patches = im2col(image, 16x16)            # [196, 768]
  out = gelu(patches @ W + b, exact)        # [196, 768]

Sharding: batch 64 -> 8 images per core; host concatenates per-core outputs.

Layout strategy: im2col is a pure permutation for stride-16 non-overlapping
patches, so the HOST performs im2col + transpose + bf16 cast and uploads
X^T in k-major chunk layout xt[p, kc, m] = X[m, 128*kc + p]. Every device
DMA is then a wide contiguous read (>=1.5 KB per partition line) -- this
removes the 192-byte-fragment im2col gather and the on-device xbar
transposes that dominated the previous version.

Matmul orientation: transposed output. For each 128-wide n-chunk,
  psum[n, m] = sum_kc W[k, n].T @ X^T[k, m]
with W chunks as the stationary operand (natural layout, uploaded
pre-chunked) and X^T as the bf16 moving operand. Benefits:
  - no bias matmuls: bias is per-PARTITION in this orientation, applied for
    free by ScalarE as gelu(psum + bias[p]) during the PSUM->SBUF pass
  - m-tiles of 392 (=1568/4) tile M exactly; lhsT is always full 128x128
  - output stored bf16 (halves store traffic); host transposes + upcasts.

Per-core schedule (m-blocks of 196/392/392/392/196):
  - Loads ride ONE HWDGE ring in exact consumption order (both rings
    share the 16 SDMA engines, so a second ring adds no bandwidth);
    the first m-block's W/X chunks are interleaved so the PE starts
    ~2 us after loads begin and paces with the DMA stream.
  - A few matmuls on a zeroed tile bridge the PE from the entry barrier
    to the first real matmul so the HAM clock gate (cold 1.2 GHz ->
    warm 2.4 GHz after ~3.4 us of sustained busy) warms during loads.
  - First m-block runs kc-outer with all 6 n-chunk PSUM groups live so
    each arriving chunk pair immediately yields 6 matmuls; later blocks
    run n6-outer, one PSUM bank per group, rotating through all 8 banks.
  - ScalarE applies exact GELU (+ per-partition bias) PSUM->SBUF bf16;
    stores overlap compute on the sync ring.
Steady-state matmul cadence measured at the 166 ns streaming floor for
N=392 bf16 (LDWEIGHTS fully hidden).
"""

import numpy as np
import ml_dtypes

import concourse.bass as bass
import concourse.tile as tile
import concourse.mybir as mybir
from concourse import bacc
from concourse.bass_utils import run_bass_kernel_spmd

P = 16
D = 768
B, H, W, C = 64, 224, 224, 3
NH = NW = 14
NPATCH = NH * NW            # 196
K = P * P * C               # 768
NCORES = 8
BPC = B // NCORES           # 8 images per core
M = BPC * NPATCH            # 1568 output rows per core
KC = K // 128               # 6 k-chunks
NC6 = D // 128              # 6 n-chunks
MT = 392                    # interior m-tile width
# m-block split: small first block so the PE can start on ~1.5 MB of
# loads instead of ~1.8, and a small last block to shorten the tail.
MBLOCKS = (196, 392, 392, 392, 196)
MOFF = (0, 196, 588, 980, 1372)

_BF16 = mybir.dt.bfloat16
_F32 = mybir.dt.float32


def _build():
    nc = bacc.Bacc("TRN2", target_bir_lowering=False, debug=False,
                   num_devices=NCORES)
    # Host-prepared layouts (see _run): all reads/writes contiguous.
    xt = nc.dram_tensor("xt", [128, KC, M], _BF16, kind="ExternalInput").ap()
    w = nc.dram_tensor("w", [128, KC, D], _BF16, kind="ExternalInput").ap()
    bias = nc.dram_tensor("bias", [128, NC6], _F32, kind="ExternalInput").ap()
    # Transposed output out[p, n6, m] = result[m, 128*n6 + p]; host unscrambles.
    out = nc.dram_tensor("out", [128, NC6, M], _BF16, kind="ExternalOutput").ap()

    with tile.TileContext(nc) as tc:
        _body(tc, xt, w, bias, out)
    nc.compile()
    return nc


def _body(tc, xt, w, bias, out):
    import contextlib
    ctx = contextlib.ExitStack()
    with ctx:
        nc = tc.nc
        singles = ctx.enter_context(tc.tile_pool(name="singles", bufs=1))
        opool = ctx.enter_context(tc.tile_pool(name="o", bufs=4))
        pspool = ctx.enter_context(tc.tile_pool(name="ps", bufs=8, space="PSUM"))

        # Loads on the sync ring in exact consumption order. The first
        # m-block's dependencies (W + 0.3 MB of X^T) are interleaved so
        # the kc-outer loop below consumes chunk k while chunk k+1 is in
        # flight; later blocks stream while the PE chews. bias goes on
        # the scalar ring (only needed by the first activation).
        bias_sb = singles.tile([128, NC6], _F32)
        nc.scalar.dma_start(out=bias_sb[:], in_=bias[:])
        w_sb = singles.tile([128, KC, D], _BF16)
        xt_sb = singles.tile([128, KC, M], _BF16)
        B0 = MBLOCKS[0]
        nc.sync.dma_start(out=w_sb[:, 0:2, :], in_=w[:, 0:2, :])
        nc.sync.dma_start(out=xt_sb[:, 0:3, 0:B0], in_=xt[:, 0:3, 0:B0])
        nc.sync.dma_start(out=w_sb[:, 2:4, :], in_=w[:, 2:4, :])
        nc.sync.dma_start(out=xt_sb[:, 3:6, 0:B0], in_=xt[:, 3:6, 0:B0])
        nc.sync.dma_start(out=w_sb[:, 4:6, :], in_=w[:, 4:6, :])
        for mb in range(1, len(MBLOCKS)):
            sl = np.s_[:, :, MOFF[mb]:MOFF[mb] + MBLOCKS[mb]]
            nc.sync.dma_start(out=xt_sb[sl], in_=xt[sl])

        # Matmuls on a zeroed tile bridge the PE continuously from the
        # entry barrier to the first real matmul so the HAM clock gate's
        # ~3.4 us busy window elapses during the load phase instead of
        # during real work.
        zeros = singles.tile([128, 384], _BF16)
        nc.vector.memset(zeros[:], 0.0)
        ps_warm = pspool.tile([128, 512], _F32, tag="ps")
        for _ in range(9):
            nc.tensor.matmul(ps_warm[:, :384], zeros[:, :128], zeros[:],
                             start=True, stop=True)

        # First m-block: kc-outer with all 6 n-chunk PSUM groups live, so
        # each arriving (w, x) chunk pair immediately yields 6 matmuls.
        ps0 = [pspool.tile([128, 512], _F32, tag="ps", name=f"ps0_{i}")
               for i in range(NC6)]
        for kc in range(KC):
            for n6 in range(NC6):
                nc.tensor.matmul(ps0[n6][:, :B0],
                                 w_sb[:, kc, n6 * 128:(n6 + 1) * 128],
                                 xt_sb[:, kc, 0:B0],
                                 start=(kc == 0), stop=(kc == KC - 1))
        for n6 in range(NC6):
            o_sb = opool.tile([128, B0], _BF16, tag="o0")
            nc.scalar.activation(o_sb[:], ps0[n6][:, :B0],
                                 mybir.ActivationFunctionType.Gelu,
                                 bias=bias_sb[:, n6:n6 + 1])
            nc.sync.dma_start(out=out[:, n6, 0:B0], in_=o_sb[:])

        # Remaining m-blocks: data is resident (or lands just ahead);
        # n6-outer keeps the activation/store pipeline finely paced.
        for mb in range(1, len(MBLOCKS)):
            m0, mw = MOFF[mb], MBLOCKS[mb]
            for n6 in range(NC6):
                ps = pspool.tile([128, 512], _F32, tag="ps")
                for kc in range(KC):
                    nc.tensor.matmul(ps[:, :mw],
                                     w_sb[:, kc, n6 * 128:(n6 + 1) * 128],
                                     xt_sb[:, kc, m0:m0 + mw],
                                     start=(kc == 0), stop=(kc == KC - 1))
                o_sb = opool.tile([128, mw], _BF16, tag=f"o{mw}")
                nc.scalar.activation(o_sb[:], ps[:, :mw],
                                     mybir.ActivationFunctionType.Gelu,
                                     bias=bias_sb[:, n6:n6 + 1])
                nc.sync.dma_start(out=out[:, n6, m0:m0 + mw], in_=o_sb[:])


_NC_CACHE = {}


def _get_nc():
    if "nc" not in _NC_CACHE:
        _NC_CACHE["nc"] = _build()
    return _NC_CACHE["nc"]


def _prep_core_inputs(image, W_proj, b_proj):
    """Host-side layout prep: im2col + transpose + bf16, all permutations."""
    image = np.asarray(image, dtype=np.float32)
    assert image.shape == (B, H, W, C)
    img_bf = image.astype(ml_dtypes.bfloat16)
    # im2col (row-major patch order, matching the reference)
    pat = img_bf.reshape(B, NH, P, NW, P, C).transpose(0, 1, 3, 2, 4, 5)
    pat = np.ascontiguousarray(pat).reshape(B, NPATCH, K)

    w_bf = np.asarray(W_proj, dtype=np.float32).astype(ml_dtypes.bfloat16)
    w_dev = np.ascontiguousarray(w_bf.reshape(KC, 128, D).transpose(1, 0, 2))
    b_dev = np.ascontiguousarray(
        np.asarray(b_proj, dtype=np.float32).reshape(NC6, 128).T)

    in_maps = []
    for c in range(NCORES):
        x = pat[c * BPC:(c + 1) * BPC].reshape(M, K)
        # xt[p, kc, m] = x[m, 128*kc + p]
        xt = np.ascontiguousarray(x.reshape(M, KC, 128).transpose(2, 1, 0))
        in_maps.append({"xt": xt, "w": w_dev, "bias": b_dev})
    return in_maps


def _run(image, W_proj, b_proj, **spmd_kwargs):
    spmd_kwargs.pop("transpose_mode", None)
    in_maps = _prep_core_inputs(image, W_proj, b_proj)
    nc = _get_nc()
    res = run_bass_kernel_spmd(nc, in_maps, core_ids=list(range(NCORES)),
                               **spmd_kwargs)
    # device layout [p, n6, m] -> [m, 128*n6+p] -> [BPC, NPATCH, D] f32
    outs = [
        np.ascontiguousarray(res.results[c]["out"].transpose(2, 1, 0))
        .astype(np.float32).reshape(BPC, NPATCH, D)
        for c in range(NCORES)
    ]
    full = np.concatenate(outs, axis=0)
    return full, res


def kernel(image, W_proj, b_proj):
    full, _ = _run(image, W_proj, b_proj)
    return full
